# revision 12
# baseline (speedup 1.0000x reference)
"""Distributed Trainium2 kernel for nn_AtomicLinear.

Reference math:
    perm = softmax((logits + gumbel) / T, axis=-1)          # [128, 128]
    rowsum = perm.sum(-1)                                   # [128]
    out = einsum('bi,oi,i->bo', x, weight, rowsum) + bias   # [4096, 512]

softmax(z, axis=-1) rows sum to 1 by construction (the reference's own
rowsum is 1 +- 1e-7 float noise), so the contraction reduces exactly to
    out = x @ weight.T + bias
which is what this kernel computes (verified < 4e-7 relative error vs
the full reference computation).

Sharding: data-parallel over the batch axis of x -- each of the 8 cores
takes a 512-row shard of x, replicates weight/bias, and produces its
512-row shard of the output. No collectives.

Per-core graph (SPMD, identical on all cores):
  - DMA in, split across both HWDGE rings: ident + x-shard on the sync
    ring; weight + bias on the scalar ring.
  - PE warm-up: zero-valued [1,128]x[1,512] float32r matmuls accumulated
    into the output PSUM banks (numerically exact +0). They have no
    input dependencies, so they fill the otherwise-idle PE window during
    input DMA and push the HAM clock gate to 8/8 before the real
    matmuls run.
  - bias is folded into the same PSUM accumulation groups via a
    ones-outer-product matmul, so the epilogue is a plain copy.
  - TensorE transposes weight/x tiles (fp32), DVE copies back as
    float32r.
  - 4 main matmuls (float32r): po[t] += xT[t].T @ wT -> [128 B, 512 OUT]
  - epilogue per tile: out_sb = copy(po[t]) (DVE), DMA out on
    alternating rings.
"""

import numpy as np

import concourse.bass as bass
import concourse.mybir as mybir
from concourse.bacc import Bacc
from concourse.bass import ts
from concourse.bass_utils import run_bass_kernel_spmd
from concourse.tile import TileContext

N_CORES = 8
B, IN, OUT = 4096, 128, 512
B_SH = B // N_CORES  # 512 rows of x per core
P = 128
NT = B_SH // P  # 4 batch tiles per core
WT = OUT // P  # 4 weight tiles
F32 = mybir.dt.float32
F32R = mybir.dt.float32r

# Zero-matmul warm-up count per output tile (front-loaded).
WARMUP = [4, 3, 2, 1]

_CACHED_NC = None


def _build():
    nc = Bacc()

    x_ext = nc.declare_dram_parameter("x", [B_SH, IN], F32, isOutput=False)
    w_ext = nc.declare_dram_parameter("weight", [OUT, IN], F32, isOutput=False)
    b_ext = nc.declare_dram_parameter("bias", [OUT], F32, isOutput=False)
    i_ext = nc.declare_dram_parameter("ident", [P, P], F32, isOutput=False)
    out_ext = nc.declare_dram_parameter("out", [B_SH, OUT], F32, isOutput=True)

    # x rows r = a*128 + p land on partition p, free block a.
    x_blk = x_ext.rearrange("(a p) i -> p a i", p=P)
    w_blk = w_ext.rearrange("(a p) i -> p a i", p=P)

    with TileContext(nc) as tc:
        with (
            tc.tile_pool(name="consts", bufs=1) as consts,
            tc.tile_pool(name="sbuf", bufs=1) as sbuf,
            tc.tile_pool(name="xtp", bufs=4) as xtp,
            tc.tile_pool(name="psum_w", bufs=1, space="PSUM") as psum_w_pool,
            tc.tile_pool(name="psum_x", bufs=2, space="PSUM") as psum_x_pool,
            tc.tile_pool(name="psum_out", bufs=4, space="PSUM") as psum_out,
            tc.tile_pool(name="outp", bufs=2) as outp,
        ):
            # ---- zero / ones constants for warm-up + bias matmuls ----
            # (memset can't target float32r; bounce through a DVE copy,
            # which is a valid fp32r rounding producer)
            zf = consts.tile([1, OUT], F32)
            nc.gpsimd.memset(zf, 0.0)
            zeros_b = consts.tile([1, OUT], F32R)
            nc.vector.tensor_copy(zeros_b, zf)
            zeros_a = zeros_b[:, :P]
            of = consts.tile([1, P], F32)
            nc.gpsimd.memset(of, 1.0)
            ones_r = consts.tile([1, P], F32R)
            nc.vector.tensor_copy(ones_r, of)

            # ---- PE warm-up: accumulate +0 into the output PSUM banks ----
            po = []
            for t in range(NT):
                pot = psum_out.tile([P, OUT], F32)
                po.append(pot)
            for k in range(max(WARMUP)):
                for t in range(NT):
                    if k < WARMUP[t]:
                        nc.tensor.matmul(
                            po[t], zeros_a, zeros_b, start=(k == 0), stop=False
                        )

            # ---- input DMAs ----
            ident = consts.tile([P, P], F32)
            nc.sync.dma_start(ident, i_ext[:, :])
            x_nat = sbuf.tile([P, NT, P], F32)
            nc.sync.dma_start(x_nat, x_blk)
            w_nat = sbuf.tile([P, WT, P], F32)
            nc.scalar.dma_start(w_nat, w_blk)
            bias_sb = consts.tile([1, OUT], F32)
            nc.scalar.dma_start(bias_sb, b_ext[None, :])
            bias_r = consts.tile([1, OUT], F32R)
            nc.vector.tensor_copy(bias_r, bias_sb)

            # ---- bias into each accumulation group ----
            for t in range(NT):
                nc.tensor.matmul(po[t], ones_r, bias_r, start=False, stop=False)

            # ---- weight transpose -> wT [IN, OUT] (float32r) ----
            psum_w = psum_w_pool.tile([P, WT * P], F32)
            for t in range(WT):
                nc.tensor.transpose(psum_w[:, ts(t, P)], w_nat[:, t, :], ident)
            wT = sbuf.tile([P, OUT], F32R)
            nc.vector.tensor_copy(wT, psum_w)

            # ---- x transpose (per-tile) + matmul + epilogue ----
            for t in range(NT):
                pxt = psum_x_pool.tile([P, P], F32)
                nc.tensor.transpose(pxt, x_nat[:, t, :], ident)
                xTt = xtp.tile([P, P], F32R)
                nc.vector.tensor_copy(xTt, pxt)
                nc.tensor.matmul(po[t], xTt, wT, start=False, stop=True)
                ot = outp.tile([P, OUT], F32)
                nc.vector.tensor_copy(ot, po[t])
                eng = nc.sync if t % 2 == 0 else nc.scalar
                eng.dma_start(out_ext[ts(t, P), :], ot)

    nc.finalize()
    return nc


def get_nc():
    global _CACHED_NC
    if _CACHED_NC is None:
        _CACHED_NC = _build()
    return _CACHED_NC


_IDENT = np.eye(P, dtype=np.float32)


def make_in_maps(x, weight, bias, logits, gumbel):
    x = np.ascontiguousarray(x, dtype=np.float32)
    weight = np.ascontiguousarray(weight, dtype=np.float32)
    bias = np.ascontiguousarray(bias, dtype=np.float32)
    return [
        {
            "x": np.ascontiguousarray(x[i * B_SH : (i + 1) * B_SH]),
            "weight": weight,
            "bias": bias,
            "ident": _IDENT,
        }
        for i in range(N_CORES)
    ]


def run(inputs, trace=False, **kwargs):
    nc = get_nc()
    in_maps = make_in_maps(**inputs)
    res = run_bass_kernel_spmd(
        nc, in_maps, core_ids=list(range(N_CORES)), trace=trace, **kwargs
    )
    out = np.concatenate(
        [np.asarray(res.results[i]["out"]) for i in range(N_CORES)], axis=0
    )
    return out.astype(np.float32), res


def kernel(**inputs):
    out, _ = run(inputs, trace=False)
    return out


# revision 14
# speedup vs baseline: 1.2729x; 1.2729x over previous
"""Distributed Trainium2 kernel for nn_AtomicLinear.

Reference math:
    perm = softmax((logits + gumbel) / T, axis=-1)          # [128, 128]
    rowsum = perm.sum(-1)                                   # [128]
    out = einsum('bi,oi,i->bo', x, weight, rowsum) + bias   # [4096, 512]

softmax(z, axis=-1) rows sum to 1 by construction (the reference's own
rowsum is 1 +- 1e-7 float noise), so the contraction reduces exactly to
    out = x @ weight.T + bias
which is what this kernel computes (verified < 4e-7 relative error vs
the full reference computation).

Sharding: data-parallel over the batch axis of x -- each of the 8 cores
takes a 512-row shard of x, replicates weight/bias, and produces its
512-row shard of the output. No collectives.

Per-core graph (SPMD, identical on all cores):
  - DMA in on both HWDGE rings: x-shard (2 halves) on sync ring,
    weight + bias on scalar ring.
  - identity built on GpSimd (memset + affine_select), bias broadcast
    [1,512] -> [128,512] via GpSimd partition_broadcast: keeps both off
    the DMA rings and off PE/DVE.
  - TensorE: 8 transposes back-to-back (keeps the PE out of its low
    p-state), then 4 float32r matmuls; DVE casts run in the gaps.
  - epilogue per tile: out_sb = psum + bias_bc (DVE), DMA out on
    alternating rings.
"""

import numpy as np

import concourse.bass as bass
import concourse.mybir as mybir
from concourse.bacc import Bacc
from concourse.bass import ts
from concourse.bass_utils import run_bass_kernel_spmd
from concourse.masks import make_identity
from concourse.tile import TileContext

N_CORES = 8
B, IN, OUT = 4096, 128, 512
B_SH = B // N_CORES  # 512 rows of x per core
P = 128
NT = B_SH // P  # 4 batch tiles per core
WT = OUT // P  # 4 weight tiles
F32 = mybir.dt.float32
F32R = mybir.dt.float32r

_CACHED_NC = None


def _build():
    nc = Bacc()

    x_ext = nc.declare_dram_parameter("x", [B_SH, IN], F32, isOutput=False)
    w_ext = nc.declare_dram_parameter("weight", [OUT, IN], F32, isOutput=False)
    b_ext = nc.declare_dram_parameter("bias", [OUT], F32, isOutput=False)
    out_ext = nc.declare_dram_parameter("out", [B_SH, OUT], F32, isOutput=True)

    # x rows r = a*128 + p land on partition p, free block a.
    x_blk = x_ext.rearrange("(a p) i -> p a i", p=P)
    w_blk = w_ext.rearrange("(a p) i -> p a i", p=P)

    with TileContext(nc) as tc:
        with (
            tc.tile_pool(name="consts", bufs=1) as consts,
            tc.tile_pool(name="sbuf", bufs=1) as sbuf,
            tc.tile_pool(name="xtp", bufs=4) as xtp,
            tc.tile_pool(name="psum_w", bufs=1, space="PSUM") as psum_w_pool,
            tc.tile_pool(name="psum_x", bufs=4, space="PSUM") as psum_x_pool,
            tc.tile_pool(name="psum_out", bufs=2, space="PSUM") as psum_out,
            tc.tile_pool(name="outp", bufs=2) as outp,
        ):
            # ---- input DMAs ----
            x_nat = sbuf.tile([P, NT, P], F32)
            nc.sync.dma_start(x_nat[:, 0:2, :], x_blk[:, 0:2, :])
            nc.sync.dma_start(x_nat[:, 2:4, :], x_blk[:, 2:4, :])
            w_nat = sbuf.tile([P, WT, P], F32)
            nc.scalar.dma_start(w_nat, w_blk)
            bias_bc = consts.tile([P, OUT], F32)
            nc.scalar.dma_start(bias_bc, b_ext[None, :].broadcast_to([P, OUT]))

            # ---- identity on GpSimd ----
            ident = consts.tile([P, P], F32)
            make_identity(nc, ident)

            # ---- 8 transposes back-to-back on PE ----
            psum_w = psum_w_pool.tile([P, WT * P], F32)
            for t in range(WT):
                nc.tensor.transpose(psum_w[:, ts(t, P)], w_nat[:, t, :], ident)
            pxts = []
            for t in range(NT):
                pxt = psum_x_pool.tile([P, P], F32)
                nc.tensor.transpose(pxt, x_nat[:, t, :], ident)
                pxts.append(pxt)

            # ---- casts (DVE) ----
            wT = sbuf.tile([P, OUT], F32R)
            nc.vector.tensor_copy(wT, psum_w)
            xTs = []
            for t in range(NT):
                xTt = xtp.tile([P, P], F32R)
                nc.vector.tensor_copy(xTt, pxts[t])
                xTs.append(xTt)

            # ---- main matmuls back-to-back + epilogue ----
            for t in range(NT):
                po = psum_out.tile([P, OUT], F32)
                nc.tensor.matmul(po, xTs[t], wT, start=True, stop=True)
                ot = outp.tile([P, OUT], F32)
                nc.vector.tensor_add(ot, po, bias_bc)
                eng = nc.sync if t % 2 == 0 else nc.scalar
                eng.dma_start(out_ext[ts(t, P), :], ot)

    nc.finalize()
    return nc


def get_nc():
    global _CACHED_NC
    if _CACHED_NC is None:
        _CACHED_NC = _build()
    return _CACHED_NC


def make_in_maps(x, weight, bias, logits, gumbel):
    x = np.ascontiguousarray(x, dtype=np.float32)
    weight = np.ascontiguousarray(weight, dtype=np.float32)
    bias = np.ascontiguousarray(bias, dtype=np.float32)
    return [
        {
            "x": np.ascontiguousarray(x[i * B_SH : (i + 1) * B_SH]),
            "weight": weight,
            "bias": bias,
        }
        for i in range(N_CORES)
    ]


def run(inputs, trace=False, **kwargs):
    nc = get_nc()
    in_maps = make_in_maps(**inputs)
    res = run_bass_kernel_spmd(
        nc, in_maps, core_ids=list(range(N_CORES)), trace=trace, **kwargs
    )
    out = np.concatenate(
        [np.asarray(res.results[i]["out"]) for i in range(N_CORES)], axis=0
    )
    return out.astype(np.float32), res


def kernel(**inputs):
    out, _ = run(inputs, trace=False)
    return out
